# revision 1
# baseline (speedup 1.0000x reference)
"""Trainium2 Bass kernel for nn_AutoregressiveMixerBlock.

Reference computation (per batch b):
  y  = LN_H(x)                                    # layer norm over H
  t  = revcumsum_N(y)                             # t[j] = sum_{i>=j} y[i]
  h  = gelu(t^T @ tok_w1 + tok_b1)                # [H, TM]
  y2 = (h @ tok_w2 + tok_b2)^T                    # [N, H]
  y3 = LN_H(y2)
  out = gelu(y3 @ ch_w1 + ch_b1) @ ch_w2 + ch_b2  # [N, H]

Key algebraic folds (exact in real arithmetic, applied on host):
  * revcumsum+matmul:  sum_j t[j,h] w1[j,m] = sum_i y[i,h] W1c[i,m]
    with W1c = cumsum(tok_w1, axis=0) -> no on-device cumsum at all.
  * LN1 gain/bias move past the token matmul:
    out1[h,m] = g[h] * (yn^T @ W1c)[h,m] + (b[h]*colsum1[m] + tok_b1[m])
  * tok_b2 and the LN2 mean both vanish by centering h^T by its
    per-row (over H) mean before the second token matmul.
  * LN2 gain/bias fold into ch_w1 / ch_b1.

Sharding: data-parallel over B across 8 cores (2 batches per core),
weights replicated.
"""

import numpy as np

B, N, H = 16, 8192, 128
TM, CM = 256, 512
EPS = 1e-5
NCORES = 8
BL = B // NCORES          # batches per core
P = 128                   # partitions
NC_TOK = N // P           # 64 token chunks of 128
NJ = N // 512             # 16 column chunks of 512
KTM = TM // P             # 2 k-chunks for the second token matmul
NCI = CM // P             # 4 chunks of the channel hidden dim

_cached = {}


def _build(nontrivial_bias1, nontrivial_cb2):
    import concourse.bass as bass
    import concourse.mybir as mybir
    import concourse.tile as tile
    from concourse import bacc
    from concourse.masks import make_identity
    import bass_rust

    F32 = mybir.dt.float32
    F32R = mybir.dt.float32r
    BF16 = mybir.dt.bfloat16
    AF = mybir.ActivationFunctionType
    ALU = mybir.AluOpType
    AX = mybir.AxisListType

    nc = bacc.Bacc()

    # ---- DRAM tensors -------------------------------------------------
    x_d = nc.dram_tensor("x", [BL, N, H], F32, kind="ExternalInput")
    w1c_d = nc.dram_tensor("w1c", [N, TM], F32R, kind="ExternalInput")
    w2_d = nc.dram_tensor("w2", [TM, N], F32R, kind="ExternalInput")
    g1_d = nc.dram_tensor("g1", [P, 1], F32, kind="ExternalInput")
    bias1_d = nc.dram_tensor("bias1", [P, TM], F32, kind="ExternalInput")
    cw1_d = nc.dram_tensor("cw1", [H, CM], BF16, kind="ExternalInput")
    cb1_d = nc.dram_tensor("cb1", [P, NCI], F32, kind="ExternalInput")
    cw2_d = nc.dram_tensor("cw2", [CM, H], BF16, kind="ExternalInput")
    cb2_d = nc.dram_tensor("cb2", [P, 1], F32, kind="ExternalInput")
    ones_d = nc.dram_tensor("ones", [P, P], F32R, kind="ExternalInput")
    out_d = nc.dram_tensor("out", [BL, H, N], F32, kind="ExternalOutput")

    # DRAM views
    x_v = [x_d[b].rearrange("(c p) h -> p c h", p=P) for b in range(BL)]
    w1c_v = w1c_d[:].rearrange("(c p) m -> p c m", p=P)
    w2_v = w2_d[:].rearrange("(k p) (j n) -> p k j n", p=P, n=512)
    cw2_v = cw2_d[:].rearrange("(ci p) h -> p ci h", p=P)
    out_v = [out_d[b] for b in range(BL)]

    act_phases = [[], [], [], []]  # ACT table-set phase buckets

    with tile.TileContext(nc) as tc:
        import contextlib
        with contextlib.ExitStack() as ctx:
            const = ctx.enter_context(tc.tile_pool(name="const", bufs=1))
            xall = ctx.enter_context(tc.tile_pool(name="xall", bufs=BL))
            stats = ctx.enter_context(tc.tile_pool(name="stats", bufs=2 * BL))
            small = ctx.enter_context(tc.tile_pool(name="small", bufs=4))
            sqp = ctx.enter_context(tc.tile_pool(name="sqp", bufs=1))
            w1cs = ctx.enter_context(tc.tile_pool(name="w1cs", bufs=4))
            w2s = ctx.enter_context(tc.tile_pool(name="w2s", bufs=6))
            sq2p = ctx.enter_context(tc.tile_pool(name="sq2p", bufs=3))
            rstdp = ctx.enter_context(tc.tile_pool(name="rstdp", bufs=3))
            g2p = ctx.enter_context(tc.tile_pool(name="g2p", bufs=2))
            outp = ctx.enter_context(tc.tile_pool(name="outp", bufs=3))

            # ---- constants -------------------------------------------
            g1_sb = const.tile([P, 1], F32)
            nc.sync.dma_start(g1_sb, g1_d[:])
            cw1_sb = const.tile([H, CM], BF16)
            nc.sync.dma_start(cw1_sb, cw1_d[:])
            cb1_sb = const.tile([P, NCI], F32)
            nc.sync.dma_start(cb1_sb, cb1_d[:])
            cw2_sb = const.tile([P, NCI, H], BF16)
            nc.sync.dma_start(cw2_sb, cw2_v)
            ones_sb = const.tile([P, P], F32R)
            nc.sync.dma_start(ones_sb, ones_d[:])
            ident = const.tile([P, P], F32)
            make_identity(nc, ident)
            if nontrivial_bias1:
                bias1_sb = const.tile([P, TM], F32)
                nc.sync.dma_start(bias1_sb, bias1_d[:])
            if nontrivial_cb2:
                cb2_sb = const.tile([P, 1], F32)
                nc.sync.dma_start(cb2_sb, cb2_d[:])
                cb2_t = small.tile([P, 1], F32, tag="cb2t")
                nc.vector.tensor_copy(cb2_t, cb2_sb)
            # pre-touch the per-partition scalar so later scalar-pointer
            # ops don't need a DMA wait of their own
            g1_t = small.tile([P, 1], F32)
            nc.vector.tensor_copy(g1_t, g1_sb)
            eps_t = const.tile([P, 1], F32)
            nc.vector.memset(eps_t, EPS)

            # ---- phase 1: LN1 stats + normalize + token matmul 1 -----
            x_sb = []
            rstd1 = []
            mu1 = []
            for b in range(BL):
                xt = xall.tile([P, NC_TOK, H], F32, tag="xall", name=f"xall{b}")
                nc.sync.dma_start(xt, x_v[b])
                x_sb.append(xt)

                sums = stats.tile([P, NC_TOK], F32, tag="st_sum")
                nc.vector.tensor_reduce(
                    out=sums, in_=xt, axis=AX.X, op=ALU.add)
                sq = sqp.tile([P, NC_TOK, H], BF16, tag="sq")
                i_sq = nc.scalar.activation(sq, xt, AF.Square)
                act_phases[0].append(i_sq)
                sumsq = stats.tile([P, NC_TOK], F32, tag="st_sumsq")
                nc.vector.tensor_reduce(
                    out=sumsq, in_=sq, axis=AX.X, op=ALU.add)

                mu = stats.tile([P, NC_TOK], F32, tag="st_mu")
                nc.vector.tensor_scalar_mul(mu, sums, 1.0 / H)
                ex2 = stats.tile([P, NC_TOK], F32, tag="st_ex2")
                nc.vector.tensor_scalar_mul(ex2, sumsq, 1.0 / H)
                musq = stats.tile([P, NC_TOK], F32, tag="st_musq")
                nc.vector.tensor_tensor(musq, mu, mu, ALU.mult)
                var = stats.tile([P, NC_TOK], F32, tag="st_var")
                nc.vector.tensor_tensor(var, ex2, musq, ALU.subtract)
                nc.vector.tensor_scalar(
                    out=var, in0=var, scalar1=EPS, scalar2=None, op0=ALU.add)
                std = stats.tile([P, NC_TOK], F32, tag="st_std")
                i_r = nc.scalar.activation(std, var, AF.Sqrt)
                act_phases[0].append(i_r)
                rst = stats.tile([P, NC_TOK], F32, tag="st_rstd")
                nc.vector.reciprocal_approx_fast(rst, std)
                rstd1.append(rst)
                mu1.append(mu)

            with (
                tc.tile_pool(name="ps1", bufs=BL, space="PSUM") as ps1,
                tc.tile_pool(name="pst", bufs=2, space="PSUM") as pst,
            ):
                psum1 = [ps1.tile([P, TM], F32, tag="ps1", name=f"ps1_{b}")
                         for b in range(BL)]
                for c in range(NC_TOK):
                    w1t = w1cs.tile([P, TM], F32R, tag="w1c")
                    nc.sync.dma_start(w1t, w1c_v[:, c, :])
                    for b in range(BL):
                        xn = small.tile([P, P], F32R, tag="xn")
                        nc.vector.tensor_scalar(
                            out=xn,
                            in0=x_sb[b][:, c, :],
                            scalar1=mu1[b][:, c:c + 1],
                            scalar2=rstd1[b][:, c:c + 1],
                            op0=ALU.subtract,
                            op1=ALU.mult,
                        )
                        nc.tensor.matmul(
                            psum1[b],
                            xn,
                            w1t,
                            start=(c == 0),
                            stop=(c == NC_TOK - 1),
                        )

                # ---- phase 2: token gelu, transpose, center --------------
                h1c = []  # per batch: list of KTM [P, P] f32r tiles
                for b in range(BL):
                    h1 = small.tile([P, TM], F32, tag="h1")
                    if nontrivial_bias1:
                        nc.vector.tensor_scalar_mul(h1, psum1[b], g1_t)
                        nc.vector.tensor_add(h1, h1, bias1_sb)
                        i_g = nc.scalar.activation(h1, h1, AF.Gelu)
                    else:
                        i_g = nc.scalar.activation(h1, psum1[b], AF.Gelu,
                                                   scale=g1_t)
                    act_phases[1].append(i_g)

                    chunks = []
                    for k in range(KTM):
                        ps_t = pst.tile([P, P], F32, tag="pst")
                        nc.tensor.transpose(ps_t, h1[:, k * P:(k + 1) * P], ident)
                        h1T = small.tile([P, P], F32, tag="h1T")
                        nc.vector.tensor_copy(h1T, ps_t)
                        hsum = small.tile([P, 1], F32, tag="hsum")
                        nc.vector.tensor_reduce(
                            out=hsum, in_=h1T, axis=AX.X, op=ALU.add)
                        hmean = small.tile([P, 1], F32, tag="hmean")
                        nc.vector.tensor_scalar_mul(hmean, hsum, 1.0 / H)
                        hc = small.tile([P, P], F32R, tag="h1c")
                        nc.vector.tensor_scalar(
                            out=hc, in0=h1T, scalar1=hmean, scalar2=None,
                            op0=ALU.subtract)
                        chunks.append(hc)
                    h1c.append(chunks)

            # ---- phase 3a: token matmul 2 + LN2 stats ----------------
            with (
                tc.tile_pool(name="ps2", bufs=3, space="PSUM") as ps2,
                tc.tile_pool(name="psv", bufs=2, space="PSUM") as psv,
            ):
                y2n = []
                for b in range(BL):
                    y2n.append(xall.tile([P, N], BF16, tag="xall", name=f"y2n{b}"))

                for j in range(NJ):
                    w2t = []
                    for k in range(KTM):
                        wt = w2s.tile([P, 512], F32R, tag="w2")
                        nc.sync.dma_start(wt, w2_v[:, k, j, :])
                        w2t.append(wt)
                    for b in range(BL):
                        p2 = ps2.tile([P, 512], F32, tag="ps2")
                        for k in range(KTM):
                            nc.tensor.matmul(
                                p2, h1c[b][k], w2t[k],
                                start=(k == 0), stop=(k == KTM - 1))
                        sq2 = sq2p.tile([P, 512], F32R, tag="sq2")
                        i_s = nc.scalar.activation(
                            sq2, p2, AF.Square, scale=float(1.0 / np.sqrt(H)))
                        act_phases[2].append(i_s)
                        vps = psv.tile([P, 512], F32, tag="psv")
                        nc.tensor.matmul(vps, ones_sb, sq2, start=True, stop=True)
                        std = rstdp.tile([P, 512], F32, tag="std")
                        i_r = nc.scalar.activation(std, vps, AF.Sqrt, bias=eps_t)
                        act_phases[2].append(i_r)
                        rstd = rstdp.tile([P, 512], F32, tag="rstd")
                        nc.vector.reciprocal_approx_fast(rstd, std)
                        nc.vector.tensor_tensor(
                            y2n[b][:, j * 512:(j + 1) * 512],
                            p2, rstd, ALU.mult)

            # ---- phase 3b: channel MLP -------------------------------
            with (
                tc.tile_pool(name="psr", bufs=1, space="PSUM") as psr,
                tc.tile_pool(name="pso", bufs=2, space="PSUM") as pso,
            ):
                for j in range(NJ):
                    for b in range(BL):
                        y2s = y2n[b][:, j * 512:(j + 1) * 512]
                        raw2 = psr.tile([P, NCI * 512], F32, tag="psr")
                        for ci in range(NCI):
                            nc.tensor.matmul(
                                raw2[:, ci * 512:(ci + 1) * 512],
                                cw1_sb[:, ci * P:(ci + 1) * P],
                                y2s, start=True, stop=True)
                        g2 = g2p.tile([P, NCI * 512], BF16, tag="g2")
                        if nontrivial_bias1:
                            # general path: per-ci bias
                            for ci in range(NCI):
                                i_g = nc.scalar.activation(
                                    g2[:, ci * 512:(ci + 1) * 512],
                                    raw2[:, ci * 512:(ci + 1) * 512],
                                    AF.Gelu, bias=cb1_sb[:, ci:ci + 1])
                                act_phases[3].append(i_g)
                        else:
                            i_g = nc.scalar.activation(g2, raw2, AF.Gelu)
                            act_phases[3].append(i_g)

                        po = pso.tile([P, 512], F32, tag="pso")
                        for ci in range(NCI):
                            nc.tensor.matmul(
                                po,
                                cw2_sb[:, ci, :],
                                g2[:, ci * 512:(ci + 1) * 512],
                                start=(ci == 0), stop=(ci == NCI - 1))
                        osb = outp.tile([P, 512], F32, tag="osb")
                        if nontrivial_cb2:
                            nc.vector.tensor_scalar(
                                out=osb, in0=po, scalar1=cb2_t, scalar2=None,
                                op0=ALU.add)
                        else:
                            nc.vector.tensor_copy(osb, po)
                        nc.sync.dma_start(
                            out_v[b][:, j * 512:(j + 1) * 512], osb)

            # ---- ACT table-set ordering edges ------------------------
            for ph in range(3):
                for f in act_phases[ph + 1]:
                    for t in act_phases[ph]:
                        bass_rust.add_dep_helper(
                            f.ins, t.ins, sync=False,
                            reason="act table set phase ordering")

    nc.compile()
    return nc


def _host_prep(inputs):
    x = np.ascontiguousarray(inputs["x"], dtype=np.float32)
    ln1_g = np.asarray(inputs["ln1_g"], np.float32)
    ln1_b = np.asarray(inputs["ln1_b"], np.float32)
    ln2_g = np.asarray(inputs["ln2_g"], np.float32)
    ln2_b = np.asarray(inputs["ln2_b"], np.float32)
    tok_w1 = np.asarray(inputs["tok_w1"], np.float32)
    tok_b1 = np.asarray(inputs["tok_b1"], np.float32)
    tok_w2 = np.asarray(inputs["tok_w2"], np.float32)
    ch_w1 = np.asarray(inputs["ch_w1"], np.float32)
    ch_b1 = np.asarray(inputs["ch_b1"], np.float32)
    ch_w2 = np.asarray(inputs["ch_w2"], np.float32)
    ch_b2 = np.asarray(inputs["ch_b2"], np.float32)

    import ml_dtypes
    w1c = np.cumsum(tok_w1, axis=0, dtype=np.float64).astype(np.float32)
    colsum1 = w1c.sum(axis=0, dtype=np.float64).astype(np.float32)
    bias1 = ln1_b[:, None] * colsum1[None, :] + tok_b1[None, :]
    cw1 = (ln2_g[:, None] * ch_w1).astype(np.float32)
    cb1 = (ch_b1 + ch_w1.T @ ln2_b).astype(np.float32)
    cw2 = ch_w2.astype(ml_dtypes.bfloat16)


    nontrivial_bias1 = bool(np.any(bias1 != 0.0) or np.any(cb1 != 0.0))
    nontrivial_cb2 = bool(np.any(ch_b2 != 0.0))

    shared = {
        "w1c": w1c,
        "w2": np.ascontiguousarray(tok_w2),
        "g1": ln1_g.reshape(P, 1).copy(),
        "bias1": np.ascontiguousarray(bias1, np.float32),
        "cw1": cw1.astype(ml_dtypes.bfloat16),
        "cb1": np.ascontiguousarray(cb1.reshape(NCI, P).T.copy()),
        "cw2": np.ascontiguousarray(cw2),
        "cb2": ch_b2.reshape(P, 1).astype(np.float32).copy(),
        "ones": np.ones((P, P), np.float32),
    }
    return x, shared, nontrivial_bias1, nontrivial_cb2


def kernel(**inputs) -> np.ndarray:
    from concourse.bass_utils import run_bass_kernel_spmd

    x, shared, nb1, nb2 = _host_prep(inputs)

    key = (nb1, nb2)
    if key not in _cached:
        _cached[key] = _build(nb1, nb2)
    nc = _cached[key]

    in_maps = []
    for c in range(NCORES):
        m = dict(shared)
        m["x"] = np.ascontiguousarray(x[c * BL:(c + 1) * BL])
        in_maps.append(m)

    res = run_bass_kernel_spmd(nc, in_maps, core_ids=list(range(NCORES)))
    out = np.concatenate(
        [r["out"].transpose(0, 2, 1) for r in res.results], axis=0)
    return np.ascontiguousarray(out, dtype=np.float32)


if __name__ == "__main__":
    rng = np.random.default_rng(0)
    ins = {
        "x": rng.standard_normal((B, N, H)).astype(np.float32),
        "ln1_g": np.ones(H, np.float32),
        "ln1_b": np.zeros(H, np.float32),
        "ln2_g": np.ones(H, np.float32),
        "ln2_b": np.zeros(H, np.float32),
        "tok_w1": (rng.standard_normal((N, TM)) * 0.02).astype(np.float32),
        "tok_b1": np.zeros(TM, np.float32),
        "tok_w2": (rng.standard_normal((TM, N)) * 0.02).astype(np.float32),
        "tok_b2": np.zeros(N, np.float32),
        "ch_w1": (rng.standard_normal((H, CM)) * 0.02).astype(np.float32),
        "ch_b1": np.zeros(CM, np.float32),
        "ch_w2": (rng.standard_normal((CM, H)) * 0.02).astype(np.float32),
        "ch_b2": np.zeros(H, np.float32),
    }
    out = kernel(**ins)
    print("out", out.shape, out.dtype)



# revision 11
# speedup vs baseline: 1.3297x; 1.3297x over previous
"""Trainium2 Bass kernel for nn_AutoregressiveMixerBlock.

Reference computation (per batch b):
  y  = LN_H(x)                                    # layer norm over H
  t  = revcumsum_N(y)                             # t[j] = sum_{i>=j} y[i]
  h  = gelu(t^T @ tok_w1 + tok_b1)                # [H, TM]
  y2 = (h @ tok_w2 + tok_b2)^T                    # [N, H]
  y3 = LN_H(y2)
  out = gelu(y3 @ ch_w1 + ch_b1) @ ch_w2 + ch_b2  # [N, H]

Algebraic folds (exact in real arithmetic, applied on host):
  * revcumsum+matmul:  sum_j t[j,h] w1[j,m] = sum_i y[i,h] W1c[i,m]
    with W1c = cumsum(tok_w1, axis=0) -> no on-device cumsum at all.
  * LN1 gain/bias move past the token matmul.
  * tok_b2 and the LN2 mean vanish by centering h^T by its per-row
    (over H) mean before the second token matmul.
  * LN2 gain/bias fold into ch_w1 / ch_b1.

v2 schedule: everything bf16 into the PE, single fully-pipelined
emission (no phase barriers): per-chunk LN1 stats + token matmul 1
stream, then a token-matmul-2/LN2 sweep and a channel-MLP sweep whose
PE/DVE work overlaps; ACT table phases (sqrt vs gelu sets) are kept
coarse via explicit ordering edges between Sqrt and Gelu instructions
only (Square/Identity/Copy live in every table set).

Sharding: data-parallel over B across 8 cores (2 batches per core),
weights replicated.
"""

import numpy as np

B, N, H = 16, 8192, 128
TM, CM = 256, 512
EPS = 1e-5
NCORES = 8
BL = B // NCORES          # batches per core
P = 128                   # partitions
NC = N // P               # 64 token chunks of 128
SG = 8                    # chunks per stats group
NG = NC // SG             # stats groups per batch
NJ = N // 512             # 16 column blocks of 512 tokens
KTM = TM // P             # 2 k-chunks for the second token matmul
NCI = CM // P             # 4 chunks of the channel hidden dim

_cached = {}


def _build(nontrivial_bias1, nontrivial_cb2):
    import concourse.bass as bass
    import concourse.mybir as mybir
    import concourse.tile as tile
    from concourse import bacc
    import bass_rust

    F32 = mybir.dt.float32
    BF16 = mybir.dt.bfloat16
    AF = mybir.ActivationFunctionType
    ALU = mybir.AluOpType
    AX = mybir.AxisListType

    nc = bacc.Bacc()

    # ---- DRAM tensors -------------------------------------------------
    # x pre-transposed on host to [BL, P, NC, H] (partition = token%128)
    x_d = nc.dram_tensor("x", [BL, P, NC, H], BF16, kind="ExternalInput")
    w1c_d = nc.dram_tensor("w1c", [P, NC, TM], BF16, kind="ExternalInput")
    w2_d = nc.dram_tensor("w2", [P, KTM, NJ, 512], BF16, kind="ExternalInput")
    g1_d = nc.dram_tensor("g1", [P, 1], F32, kind="ExternalInput")
    bias1_d = nc.dram_tensor("bias1", [P, TM], F32, kind="ExternalInput")
    cw1_d = nc.dram_tensor("cw1", [H, CM], BF16, kind="ExternalInput")
    cb1_d = nc.dram_tensor("cb1", [P, NCI], F32, kind="ExternalInput")
    cw2_d = nc.dram_tensor("cw2", [P, NCI, H], BF16, kind="ExternalInput")
    cb2_d = nc.dram_tensor("cb2", [P, 1], F32, kind="ExternalInput")
    out_d = nc.dram_tensor("out", [BL, H, N], F32, kind="ExternalOutput")

    # ACT table ordering buckets: only Sqrt and Gelu instructions
    # constrain the table set (Square/Identity/Copy are in every set).
    ph_sqrt1, ph_gelu1, ph_sqrt2, ph_gelu2 = [], [], [], []

    with tile.TileContext(nc) as tc:
        import contextlib
        with contextlib.ExitStack() as ctx:
            const = ctx.enter_context(tc.tile_pool(name="const", bufs=1))
            bigp = ctx.enter_context(tc.tile_pool(name="bigp", bufs=BL))
            w1cp = ctx.enter_context(tc.tile_pool(name="w1cp", bufs=1))
            w2p = ctx.enter_context(tc.tile_pool(name="w2p", bufs=1))
            statp = ctx.enter_context(tc.tile_pool(name="statp", bufs=2))
            murp = ctx.enter_context(tc.tile_pool(name="murp", bufs=16))
            xnp = ctx.enter_context(tc.tile_pool(name="xnp", bufs=6))
            sqscr = ctx.enter_context(tc.tile_pool(name="sqscr", bufs=2))
            h1p = ctx.enter_context(tc.tile_pool(name="h1p", bufs=2))
            h1cp = ctx.enter_context(tc.tile_pool(name="h1cp", bufs=4))
            sq2p = ctx.enter_context(tc.tile_pool(name="sq2p", bufs=3))
            stdp = ctx.enter_context(tc.tile_pool(name="stdp", bufs=2))
            rstdp = ctx.enter_context(tc.tile_pool(name="rstdp", bufs=2))
            g2p = ctx.enter_context(tc.tile_pool(name="g2p", bufs=3))
            outp = ctx.enter_context(tc.tile_pool(name="outp", bufs=3))
            # PSUM: 1 + 2 + 1 + 2 + 2 = 8 banks
            ps1p = ctx.enter_context(
                tc.tile_pool(name="ps1p", bufs=1, space="PSUM"))
            p2p = ctx.enter_context(
                tc.tile_pool(name="p2p", bufs=2, space="PSUM"))
            vpsp = ctx.enter_context(
                tc.tile_pool(name="vpsp", bufs=1, space="PSUM"))
            r2p = ctx.enter_context(
                tc.tile_pool(name="r2p", bufs=2, space="PSUM"))
            pop = ctx.enter_context(
                tc.tile_pool(name="pop", bufs=2, space="PSUM"))

            # ---- constants -------------------------------------------
            g1_sb = const.tile([P, 1], F32)
            nc.sync.dma_start(g1_sb, g1_d[:])
            cw1_sb = const.tile([H, CM], BF16)
            nc.sync.dma_start(cw1_sb, cw1_d[:])
            cw2_sb = const.tile([P, NCI, H], BF16)
            nc.sync.dma_start(cw2_sb, cw2_d[:])
            if nontrivial_bias1:
                bias1_sb = const.tile([P, TM], F32)
                nc.sync.dma_start(bias1_sb, bias1_d[:])
                cb1_sb = const.tile([P, NCI], F32)
                nc.sync.dma_start(cb1_sb, cb1_d[:])
            if nontrivial_cb2:
                cb2_sb = const.tile([P, 1], F32)
                nc.sync.dma_start(cb2_sb, cb2_d[:])
                cb2_t = const.tile([P, 1], F32)
                nc.vector.tensor_copy(cb2_t, cb2_sb)
            g1_t = const.tile([P, 1], F32)
            nc.vector.tensor_copy(g1_t, g1_sb)
            eps_t = const.tile([P, 1], F32)
            nc.vector.memset(eps_t, EPS)
            ones_sb = const.tile([P, P], BF16)
            nc.vector.memset(ones_sb, 1.0)

            # ---- bulk weight/input DMAs (quartered for early start) ---
            x_sb = []
            for b in range(BL):
                xt = bigp.tile([P, NC, H], BF16, tag="big", name=f"x{b}")
                x_sb.append(xt)
            w1c_sb = w1cp.tile([P, NC, TM], BF16)
            w2_sb = w2p.tile([P, KTM, NJ, 512], BF16)
            NQ = 4
            qc = NC // NQ
            for q in range(NQ):
                sl = slice(q * qc, (q + 1) * qc)
                for b in range(BL):
                    nc.sync.dma_start(x_sb[b][:, sl, :], x_d[b][:, sl, :])
                nc.sync.dma_start(w1c_sb[:, sl, :], w1c_d[:, sl, :])
            for k in range(KTM):
                nc.sync.dma_start(w2_sb[:, k], w2_d[:, k])

            # ---- part 1: LN1 stats + normalize + token matmul 1 ------
            # Stats for both batches are emitted interleaved per group so
            # every LN1 Sqrt precedes every token Gelu on the ACT queue
            # (one act-table switch). The matmul accumulation itself runs
            # batch-sequential through a single PSUM bank: psum1(b) is
            # freed by gelu1(b) before batch b+1's accumulation rotates in.
            mus, rstds = {}, {}

            def emit_stats(b, g):
                xg = x_sb[b][:, g * SG:(g + 1) * SG, :]
                sums = statp.tile([P, SG], F32, tag="sums")
                nc.vector.tensor_reduce(
                    out=sums, in_=xg, axis=AX.X, op=ALU.add)
                sumsq = statp.tile([P, SG], F32, tag="sumsq")
                for i in range(SG):
                    scr = sqscr.tile([P, H], BF16, tag="scr")
                    nc.scalar.activation(
                        scr, x_sb[b][:, g * SG + i, :], AF.Square,
                        accum_out=sumsq[:, i:i + 1])
                mu = murp.tile([P, SG], F32, tag="mu")
                nc.vector.tensor_scalar_mul(mu, sums, 1.0 / H)
                ex2 = statp.tile([P, SG], F32, tag="ex2")
                nc.vector.tensor_scalar_mul(ex2, sumsq, 1.0 / H)
                musq = statp.tile([P, SG], F32, tag="musq")
                nc.vector.tensor_tensor(musq, mu, mu, ALU.mult)
                var = statp.tile([P, SG], F32, tag="var")
                nc.vector.tensor_tensor(var, ex2, musq, ALU.subtract)
                nc.vector.tensor_scalar(
                    out=var, in0=var, scalar1=EPS, scalar2=None,
                    op0=ALU.add)
                std = statp.tile([P, SG], F32, tag="std")
                i_r = nc.scalar.activation(std, var, AF.Sqrt)
                ph_sqrt1.append(i_r)
                rstd = murp.tile([P, SG], F32, tag="rstd")
                nc.vector.reciprocal_approx_fast(rstd, std)
                mus[(b, g)] = mu
                rstds[(b, g)] = rstd

            h1c = []
            for b in range(BL):
                psum1 = ps1p.tile([P, TM], F32, tag="ps1", name=f"ps1_{b}")
                for g in range(NG):
                    if b == 0:
                        for bb in range(BL):
                            emit_stats(bb, g)
                    mu, rstd = mus[(b, g)], rstds[(b, g)]
                    for i in range(SG):
                        c = g * SG + i
                        xn = xnp.tile([P, H], BF16, tag="xn")
                        nc.vector.tensor_scalar(
                            out=xn,
                            in0=x_sb[b][:, c, :],
                            scalar1=mu[:, i:i + 1],
                            scalar2=rstd[:, i:i + 1],
                            op0=ALU.subtract,
                            op1=ALU.mult,
                        )
                        nc.tensor.matmul(
                            psum1, xn, w1c_sb[:, c, :],
                            start=(c == 0), stop=(c == NC - 1))

                # ---- token gelu, transpose, center (frees psum1) -----
                h1 = h1p.tile([P, TM], BF16, tag="h1")
                if nontrivial_bias1:
                    h1f = h1p.tile([P, TM], F32, tag="h1f")
                    nc.vector.tensor_scalar_mul(h1f, psum1, g1_t)
                    nc.vector.tensor_add(h1f, h1f, bias1_sb)
                    i_g = nc.scalar.activation(h1, h1f, AF.Gelu)
                else:
                    i_g = nc.scalar.activation(h1, psum1, AF.Gelu,
                                               scale=g1_t)
                ph_gelu1.append(i_g)
                chunks = []
                for k in range(KTM):
                    h1T = h1p.tile([P, P], BF16, tag="h1T")
                    nc.sync.dma_start_transpose(
                        h1T, h1[:, k * P:(k + 1) * P])
                    hsum = statp.tile([P, 1], F32, tag="hsum")
                    nc.vector.tensor_reduce(
                        out=hsum, in_=h1T, axis=AX.X, op=ALU.add)
                    hmean = statp.tile([P, 1], F32, tag="hmean")
                    nc.vector.tensor_scalar_mul(hmean, hsum, 1.0 / H)
                    hc = h1cp.tile([P, P], BF16, tag="h1c")
                    nc.vector.tensor_scalar(
                        out=hc, in0=h1T, scalar1=hmean, scalar2=None,
                        op0=ALU.subtract)
                    chunks.append(hc)
                h1c.append(chunks)

            # ---- sweep 1: token matmul 2 + LN2 -> y2n ----------------
            y2n = []
            for b in range(BL):
                y2n.append(bigp.tile([P, N], BF16, tag="big",
                                     name=f"y2n{b}"))
            for j in range(NJ):
                for b in range(BL):
                    p2 = p2p.tile([P, 512], F32, tag="p2")
                    nc.tensor.matmul(p2, h1c[b][0], w2_sb[:, 0, j, :],
                                     start=True, stop=False)
                    nc.tensor.matmul(p2, h1c[b][1], w2_sb[:, 1, j, :],
                                     start=False, stop=True)
                    sq2 = sq2p.tile([P, 512], BF16, tag="sq2")
                    nc.scalar.activation(
                        sq2, p2, AF.Square, scale=float(1.0 / np.sqrt(H)))
                    vps = vpsp.tile([P, 512], F32, tag="vps")
                    nc.tensor.matmul(vps, ones_sb, sq2,
                                     start=True, stop=True)
                    std = stdp.tile([P, 512], F32, tag="std")
                    i_r = nc.scalar.activation(std, vps, AF.Sqrt,
                                               bias=eps_t)
                    ph_sqrt2.append(i_r)
                    rstd = rstdp.tile([P, 512], F32, tag="rstd")
                    nc.vector.reciprocal_approx_fast(rstd, std)
                    nc.vector.tensor_tensor(
                        y2n[b][:, j * 512:(j + 1) * 512],
                        p2, rstd, ALU.mult)

            # ---- sweep 2: channel MLP --------------------------------
            for j in range(NJ):
                for b in range(BL):
                    y2s = y2n[b][:, j * 512:(j + 1) * 512]
                    po = pop.tile([P, 512], F32, tag="po")
                    for ci in range(NCI):
                        raw2 = r2p.tile([P, 512], F32, tag="raw2")
                        nc.tensor.matmul(
                            raw2, cw1_sb[:, ci * P:(ci + 1) * P], y2s,
                            start=True, stop=True)
                        g2 = g2p.tile([P, 512], BF16, tag="g2")
                        if nontrivial_bias1:
                            i_g = nc.scalar.activation(
                                g2, raw2, AF.Gelu,
                                bias=cb1_sb[:, ci:ci + 1])
                        else:
                            i_g = nc.scalar.activation(g2, raw2, AF.Gelu)
                        ph_gelu2.append(i_g)
                        nc.tensor.matmul(
                            po, cw2_sb[:, ci, :], g2,
                            start=(ci == 0), stop=(ci == NCI - 1))
                    osb = outp.tile([P, 512], F32, tag="osb")
                    if nontrivial_cb2:
                        nc.vector.tensor_scalar(
                            out=osb, in0=po, scalar1=cb2_t, scalar2=None,
                            op0=ALU.add)
                    else:
                        nc.vector.tensor_copy(osb, po)
                    nc.sync.dma_start(
                        out_d[b][:, j * 512:(j + 1) * 512], osb)

            # ---- ACT table-set ordering edges ------------------------
            for prev, nxt in ((ph_sqrt1, ph_gelu1), (ph_gelu1, ph_sqrt2),
                              (ph_sqrt2, ph_gelu2)):
                for f in nxt:
                    for t in prev:
                        bass_rust.add_dep_helper(
                            f.ins, t.ins, sync=False,
                            reason="act table set phase ordering")

    nc.compile()
    return nc


def _host_prep(inputs):
    import ml_dtypes
    BF = ml_dtypes.bfloat16

    x = np.asarray(inputs["x"], dtype=np.float32)
    ln1_g = np.asarray(inputs["ln1_g"], np.float32)
    ln1_b = np.asarray(inputs["ln1_b"], np.float32)
    ln2_g = np.asarray(inputs["ln2_g"], np.float32)
    ln2_b = np.asarray(inputs["ln2_b"], np.float32)
    tok_w1 = np.asarray(inputs["tok_w1"], np.float32)
    tok_b1 = np.asarray(inputs["tok_b1"], np.float32)
    tok_w2 = np.asarray(inputs["tok_w2"], np.float32)
    ch_w1 = np.asarray(inputs["ch_w1"], np.float32)
    ch_b1 = np.asarray(inputs["ch_b1"], np.float32)
    ch_w2 = np.asarray(inputs["ch_w2"], np.float32)
    ch_b2 = np.asarray(inputs["ch_b2"], np.float32)

    # x: [B, N, H] -> [B, P, NC, H] (partition-major chunks)
    xp = np.ascontiguousarray(
        x.reshape(B, NC, P, H).transpose(0, 2, 1, 3)).astype(BF)

    w1c = np.cumsum(tok_w1, axis=0, dtype=np.float64).astype(np.float32)
    colsum1 = w1c.sum(axis=0, dtype=np.float64).astype(np.float32)
    bias1 = ln1_b[:, None] * colsum1[None, :] + tok_b1[None, :]
    w1cp = np.ascontiguousarray(
        w1c.reshape(NC, P, TM).transpose(1, 0, 2)).astype(BF)
    w2p = np.ascontiguousarray(
        tok_w2.reshape(KTM, P, NJ, 512).transpose(1, 0, 2, 3)).astype(BF)
    cw1 = (ln2_g[:, None] * ch_w1).astype(BF)
    cb1 = (ch_b1 + ch_w1.T @ ln2_b).astype(np.float32)
    cw2p = np.ascontiguousarray(
        ch_w2.reshape(NCI, P, H).transpose(1, 0, 2)).astype(BF)

    nontrivial_bias1 = bool(np.any(bias1 != 0.0) or np.any(cb1 != 0.0))
    nontrivial_cb2 = bool(np.any(ch_b2 != 0.0))

    shared = {
        "w1c": w1cp,
        "w2": w2p,
        "g1": ln1_g.reshape(P, 1).copy(),
        "bias1": np.ascontiguousarray(bias1, np.float32),
        "cw1": np.ascontiguousarray(cw1),
        "cb1": np.ascontiguousarray(cb1.reshape(NCI, P).T.copy()),
        "cw2": cw2p,
        "cb2": ch_b2.reshape(P, 1).astype(np.float32).copy(),
    }
    return xp, shared, nontrivial_bias1, nontrivial_cb2


def kernel(**inputs) -> np.ndarray:
    from concourse.bass_utils import run_bass_kernel_spmd

    x, shared, nb1, nb2 = _host_prep(inputs)

    key = (nb1, nb2)
    if key not in _cached:
        _cached[key] = _build(nb1, nb2)
    nc = _cached[key]

    in_maps = []
    for c in range(NCORES):
        m = dict(shared)
        m["x"] = np.ascontiguousarray(x[c * BL:(c + 1) * BL])
        in_maps.append(m)

    res = run_bass_kernel_spmd(nc, in_maps, core_ids=list(range(NCORES)))
    out = np.concatenate(
        [r["out"].transpose(0, 2, 1) for r in res.results], axis=0)
    return np.ascontiguousarray(out, dtype=np.float32)


if __name__ == "__main__":
    rng = np.random.default_rng(0)
    ins = {
        "x": rng.standard_normal((B, N, H)).astype(np.float32),
        "ln1_g": np.ones(H, np.float32),
        "ln1_b": np.zeros(H, np.float32),
        "ln2_g": np.ones(H, np.float32),
        "ln2_b": np.zeros(H, np.float32),
        "tok_w1": (rng.standard_normal((N, TM)) * 0.02).astype(np.float32),
        "tok_b1": np.zeros(TM, np.float32),
        "tok_w2": (rng.standard_normal((TM, N)) * 0.02).astype(np.float32),
        "tok_b2": np.zeros(N, np.float32),
        "ch_w1": (rng.standard_normal((H, CM)) * 0.02).astype(np.float32),
        "ch_b1": np.zeros(CM, np.float32),
        "ch_w2": (rng.standard_normal((CM, H)) * 0.02).astype(np.float32),
        "ch_b2": np.zeros(H, np.float32),
    }
    out = kernel(**ins)
    print("out", out.shape, out.dtype)
